# revision 18
# baseline (speedup 1.0000x reference)
"""GAT layer (nn_CustomGATLayer) on 8 Trainium2 NeuronCores.

Strategy (per sharding hint): shard rows of the NxN attention matrix across
8 cores; each core owns N/8=1024 query nodes and holds Wh of all N key nodes
replicated.  Per core, scores are computed directly in transposed [key j,
query i] layout so the attention @ Wh matmul needs no on-device transposes:

  r[j,i]  = lrelu(madj[j,i] + s2[j] + s1[i])   (ONE fused custom DVE op:
                                                t = Src0+Src1+C0; max(t, a*t))
  p[j,i]  = exp(r)                             (ACT Exp -> float32r)
  acc[i,:] += p[:,iblk].T @ Wh                 (PE, fp32r, 2 query blocks per
                                                PSUM bank)
  den[i]  += p[:,iblk].T @ [1,1]               (PE, packed 1-bank accumulator)
  out[i,f] = acc[i,f] / den[i]

madj is a host-prepared additive mask in fp16: 0 where the (self-loop added)
adjacency is nonzero, -512 elsewhere, so exp(lrelu(t-512)) ~ e^-100 = 0,
matching the reference's hard masking.  Inputs are rolled per-core so every
core runs an identical program (core c's own rows sit first in its local node
order; sums over keys are permutation invariant).  Wh production (phase 1)
and score/accumulate work are emitted interleaved per 1024-node segment so
DMA, PE, ACT and DVE overlap from the start.

x/W/a inputs are fed as fp16 (PE matmuls run 1 col/cycle, DMA halved); all
DVE score traffic is fp16; Wh is kept f32r (fp32 bits) for the accumulation.
"""
import numpy as np
import ml_dtypes
from contextlib import ExitStack

import concourse.bacc as bacc
import concourse.mybir as mybir
import concourse.tile as tile
from concourse.bass_utils import run_bass_kernel_spmd

F32 = mybir.dt.float32
F32R = mybir.dt.float32r
F16 = mybir.dt.float16
AF = mybir.ActivationFunctionType
ALU = mybir.AluOpType

N = 8192
F = 256
NCORES = 8
R = N // NCORES          # 1024 query rows per core
CH = N // 128            # 64 key chunks of 128
IB = R // 128            # 8 query blocks of 128
SEG = 8                  # phase-1 segments (1024 nodes each)
NBS = CH // SEG          # key chunks per segment
MB = 4                   # key chunks per madj DMA batch
ALPHA = 0.2
BIG = 512.0

# --- tuning knobs ---
# per-pair score mode: "F" fused DVE op, "S" split DVE ops, "X" DVE add +
# ACT prelu.  Default all fused.
PAIR_MODE = ["F"] * (CH // 2)
WHE_DVE = frozenset()     # whe pair-copy indices routed to DVE (rest ACT)
NORM_ON_ACT = False
DEBUG_DUMPS = False       # add intermediate ExternalOutputs for debugging
# timing-only ablations (break numerics, keep instruction mix realistic):
#   no_den, no_acc, no_dve (scores constant), no_exp (p via DMA), no_madj
ABLATE = frozenset()


def _register_score_op():
    """Register the fused score op: out = lrelu(in0 + in1 + s0) with slope s1.

    in0: madj chunk [128, R] fp16; in1: s1 broadcast [128, R] fp16;
    s0: per-partition s2 [128, 1] fp32 AP; s1: leak slope literal.
    DVE ALUs run fp32 internally; only the output store rounds to fp16.
    """
    import concourse.dve_ops as dve_ops
    from concourse.dve_ops import DveOp, DveOpSpec, OPS
    from concourse.dve_spec import Spec, Src0, Src1, C0, C1, maxx, lower

    name = "GAT_SCORE_LRELU_ANT"
    if name in dve_ops._SUB_OPCODE_FOR_NAME:
        return next(op for op in OPS if op.name == name)

    t = Src0 + Src1 + C0
    spec = Spec(
        body=maxx(t, t * C1),
        reference=lambda in0, in1, s0, s1: np.maximum(
            in0 + in1 + s0, (in0 + in1 + s0) * s1
        ),
    )
    sub = dve_ops._CUSTOM_DVE_ROW_BASE + len(OPS)
    shas = {}
    for ver in ("v3", "v4"):
        try:
            tmp = DveOpSpec(name=name, opcode=sub, uops=lower(spec, ver=ver),
                            rd1_en=True)
            shas[ver] = tmp.sha(ver)
        except Exception:
            pass
    op = DveOp(name, spec, subdim=False, uops_sha=shas)
    OPS.append(op)
    dve_ops._SUB_OPCODE_FOR_NAME[name] = sub
    dve_ops.CUSTOM_DVE_SPECS[name] = spec
    return op


SCORE_OP = _register_score_op()


def _build(repeat=1):
    nc = bacc.Bacc("TRN2", target_bir_lowering=False, debug=False)
    xT = nc.dram_tensor("xT", [F, N], F16, kind="ExternalInput").ap()
    W = nc.dram_tensor("W", [F, F], F16, kind="ExternalInput").ap()
    WT = nc.dram_tensor("WT", [F, F], F16, kind="ExternalInput").ap()
    a12 = nc.dram_tensor("a12", [F, 2], F16, kind="ExternalInput").ap()
    madjT = nc.dram_tensor("madjT", [N, R], F16, kind="ExternalInput").ap()
    out = nc.dram_tensor("out", [R, F], F32, kind="ExternalOutput").ap()
    s1d = nc.dram_tensor("s1d", [R], F32).ap()  # bounce for s1 broadcast
    if "no_exp" in ABLATE:
        pconst = nc.dram_tensor("pconst", [128, 2 * R], F32R).ap()
    if DEBUG_DUMPS:
        dbg_s12 = nc.dram_tensor("dbg_s12", [128, CH * 2], F32,
                                 kind="ExternalOutput").ap()
        dbg_s1b = nc.dram_tensor("dbg_s1b", [128, R], F16,
                                 kind="ExternalOutput").ap()
        dbg_r = nc.dram_tensor("dbg_r", [128, 2, R], F32,
                               kind="ExternalOutput").ap()
        dbg_p = nc.dram_tensor("dbg_p", [128, 2, R], F32R,
                               kind="ExternalOutput").ap()
        dbg_pall = nc.dram_tensor("dbg_pall", [128, CH, R], F32R,
                                  kind="ExternalOutput").ap()
        dbg_whe = nc.dram_tensor("dbg_whe", [128, 2 * F], F32R,
                                 kind="ExternalOutput").ap()
        dbg_den = nc.dram_tensor("dbg_den", [128, 2 * IB], F32,
                                 kind="ExternalOutput").ap()

    with tile.TileContext(nc) as tc, ExitStack() as ctx:
        persist = ctx.enter_context(tc.tile_pool(name="persist", bufs=1))
        whe = persist.tile([128, CH * F], F32R, tag="whe")
        s1bh = persist.tile([128, R], F16, tag="s1bh")            # s1 bcast f16
        s1bf = persist.tile([128, R], F32, tag="s1bf")            # s1 bcast f32
        s12sb = persist.tile([128, CH * 2], F32, tag="s12sb")     # (s1,s2)/chunk
        wr0 = persist.tile([128, F], F16, tag="wr0")
        wr1 = persist.tile([128, F], F16, tag="wr1")
        wt0 = persist.tile([128, F], F16, tag="wt0")
        wt1 = persist.tile([128, F], F16, tag="wt1")
        a12t = persist.tile([128, 2, 2], F16, tag="a12t")
        va0 = persist.tile([128, 2], F16, tag="va0")
        va1 = persist.tile([128, 2], F16, tag="va1")
        ones = persist.tile([128, 2], F32, tag="ones")
        onesr = persist.tile([128, 2], F32R, tag="onesr")
        zh = persist.tile([128, 2 * F], F16, tag="zh")
        s1row = persist.tile([1, R], F32, tag="s1row")

        nc.sync.dma_start(wr0[:], W[0:128, :])
        nc.sync.dma_start(wr1[:], W[128:256, :])
        nc.sync.dma_start(wt0[:], WT[0:128, :])
        nc.sync.dma_start(wt1[:], WT[128:256, :])
        nc.sync.dma_start(a12t[:, 0, :], a12[0:128, :])
        nc.sync.dma_start(a12t[:, 1, :], a12[128:256, :])
        nc.vector.memset(ones[:], 1.0)
        nc.scalar.copy(onesr[:], ones[:])
        nc.vector.memset(zh[:], 0.0)

        # PSUM budget (8 banks): 4x acc pairs + 1 den + 2 whps + 1 s12/va
        psum = ctx.enter_context(tc.tile_pool(name="psum", bufs=1, space="PSUM"))
        accs = [psum.tile([128, 2 * F], F32, tag=f"acc{i}", name=f"acc{i}")
                for i in range(IB // 2)]
        den = psum.tile([128, 2 * IB], F32, tag="den")
        s12ps = psum.tile([128, CH * 2 + 2], F32, tag="s12ps")
        vps = s12ps[:, CH * 2:CH * 2 + 2]
        whps_pool = ctx.enter_context(
            tc.tile_pool(name="whps", bufs=2, space="PSUM"))

        xpool = ctx.enter_context(tc.tile_pool(name="xstage", bufs=2))
        mpool = ctx.enter_context(tc.tile_pool(name="madj", bufs=3))
        if "no_dve" in ABLATE:
            rfix = persist.tile([128, 2, R], F32, tag="rfix")
            nc.vector.memset(rfix[:], -1.0)
        if "no_madj" in ABLATE:
            mfix = persist.tile([128, MB, R], F16, tag="mfix")
            nc.vector.memset(mfix[:], 0.0)
        qpool = ctx.enter_context(tc.tile_pool(name="q", bufs=2))
        rpool = ctx.enter_context(tc.tile_pool(name="r", bufs=2))
        ppool = ctx.enter_context(tc.tile_pool(name="p", bufs=2))
        opool = ctx.enter_context(tc.tile_pool(name="o", bufs=2))
        rcpool = ctx.enter_context(tc.tile_pool(name="rc", bufs=2))

        for _rep in range(repeat):
          # va = W @ a  (lhsT = W^T); f16 in, psum f32, staged back as f16
          wts = (wt0, wt1)
          for kb, va in enumerate((va0, va1)):
            for fc in range(2):
                nc.tensor.matmul(vps, wts[fc][:, kb * 128:(kb + 1) * 128],
                                 a12t[:, fc, :], start=(fc == 0), stop=(fc == 1))
            nc.scalar.copy(va[:], vps)

          for t_acc in accs:
            nc.tensor.matmul(t_acc[:], wr0[:, 0:128], zh[:],
                             start=True, stop=False, skip_group_check=True)
          nc.tensor.matmul(den[:], wr0[:, 0:128], zh[:, 0:2 * IB],
                           start=True, stop=False, skip_group_check=True)

          s12v = s12sb[:].rearrange("p (c t) -> p c t", t=2)
          madj_tiles = {}

          def phase1_segment(s):
            """Load xT segment (f16), compute s12 + Wh chunks."""
            lo = s * R
            xk0 = xpool.tile([128, R], F16, tag="xk0", name="xk0")
            nc.sync.dma_start(xk0[:], xT[0:128, lo:lo + R])
            xk1 = xpool.tile([128, R], F16, tag="xk1", name="xk1")
            nc.sync.dma_start(xk1[:], xT[128:256, lo:lo + R])
            for j in range(0, NBS, 2):
                whps = whps_pool.tile([128, 2 * F], F32, tag="whps",
                                      name="whps")
                for jj in range(2):
                    nb = s * NBS + j + jj
                    c0 = (j + jj) * 128
                    nc.tensor.matmul(s12ps[:, nb * 2:nb * 2 + 2],
                                     xk0[:, c0:c0 + 128], va0[:],
                                     start=True, stop=False)
                    nc.tensor.matmul(s12ps[:, nb * 2:nb * 2 + 2],
                                     xk1[:, c0:c0 + 128], va1[:],
                                     start=False, stop=True)
                    dst = whps[:, jj * F:(jj + 1) * F]
                    nc.tensor.matmul(dst, xk0[:, c0:c0 + 128],
                                     wr0[:], start=True, stop=False)
                    nc.tensor.matmul(dst, xk1[:, c0:c0 + 128],
                                     wr1[:], start=False, stop=True)
                nb0 = s * NBS + j
                wdst = whe[:, nb0 * F:(nb0 + 2) * F]
                if (nb0 // 2) in WHE_DVE:
                    nc.vector.tensor_copy(wdst, whps[:])
                else:
                    nc.scalar.copy(wdst, whps[:])
            # stage s2 (and s1) of this segment to SBUF
            nc.vector.tensor_copy(s12sb[:, lo // 64:lo // 64 + 2 * NBS],
                                  s12ps[:, lo // 64:lo // 64 + 2 * NBS])

          def madj_load(g):
            mt = mpool.tile([128, MB, R], F16, tag="madj", name="madj")
            src = madjT.rearrange("(c p) r -> p c r", p=128)[:, g * MB:(g + 1) * MB, :]
            nc.sync.dma_start(mt[:], src)
            madj_tiles[g] = mt

          def main_pair(g):
            c0 = 2 * g
            mode = PAIR_MODE[g]
            if "no_madj" not in ABLATE:
                for h in range(2):
                    if (c0 + h) % MB == 0:
                        madj_load((c0 + h) // MB)
            r = (rfix if "no_dve" in ABLATE else
                 rpool.tile([128, 2, R], F32, tag="r", name="r"))
            for h in range(2):
                if "no_dve" in ABLATE:
                    break
                c = c0 + h
                mt = (mfix[:, c % MB, :] if "no_madj" in ABLATE else
                      madj_tiles[c // MB][:, c % MB, :])
                s2ap = s12v[:, c, 1:2]
                if mode == "F":
                    nc.vector._custom_dve(SCORE_OP, out=r[:, h, :], in0=mt,
                                          in1=s1bh[:], s0=s2ap, s1=ALPHA)
                elif mode == "S":
                    q = qpool.tile([128, 2, R], F16, tag="q", name="q")
                    nc.vector.tensor_scalar_add(q[:, h, :], mt, s2ap)
                    nc.vector.tensor_tensor(q[:, h, :], q[:, h, :], s1bh[:],
                                            op=ALU.add)
                    nc.vector.tensor_scalar_mul(r[:, h, :], q[:, h, :], ALPHA)
                    nc.vector.tensor_tensor(r[:, h, :], q[:, h, :], r[:, h, :],
                                            op=ALU.max)
                else:  # "X": DVE add + ACT prelu (s2 folded into bias)
                    q = qpool.tile([128, 2, R], F16, tag="q", name="q")
                    nc.vector.tensor_tensor(q[:, h, :], mt, s1bh[:],
                                            op=ALU.add)
                    nc.scalar.activation(r[:, h, :], q[:, h, :], AF.Prelu,
                                         bias=s2ap, scale=1.0, alpha=ALPHA)
            p = ppool.tile([128, 2, R], F32R, tag="p", name="p")
            if "no_exp" in ABLATE:
                nc.sync.dma_start(p[:].rearrange("p a b -> p (a b)"), pconst)
            else:
                nc.scalar.activation(p[:].rearrange("p a b -> p (a b)"),
                                     r[:].rearrange("p a b -> p (a b)"), AF.Exp)
            if DEBUG_DUMPS and g == 0:
                nc.sync.dma_start(dbg_r, r[:])
                nc.sync.dma_start(dbg_p, p[:])
            if DEBUG_DUMPS:
                nc.sync.dma_start(dbg_pall[:, 2 * g:2 * g + 2, :], p[:])
            for h in range(2):
                c = c0 + h
                for ib in range(IB):
                    lhsT = p[:, h, ib * 128:(ib + 1) * 128]
                    if "no_acc" not in ABLATE:
                        nc.tensor.matmul(
                            accs[ib // 2][:, (ib % 2) * F:(ib % 2 + 1) * F],
                            lhsT, whe[:, c * F:(c + 1) * F],
                            start=False, stop=(c == CH - 1),
                            skip_group_check=True)
                    if "no_den" not in ABLATE:
                        nc.tensor.matmul(den[:, 2 * ib:2 * ib + 2], lhsT,
                                         onesr[:],
                                         start=False, stop=(c == CH - 1),
                                         skip_group_check=True)

          # segment 0 first; s1 broadcast depends only on it
          phase1_segment(0)
          nc.sync.dma_start(s1d.rearrange("(c p) -> p c", p=128), s12v[:, 0:IB, 0])
          nc.sync.dma_start(s1row[:], s1d.rearrange("(o r) -> o r", o=1))
          nc.gpsimd.partition_broadcast(s1bf[:], s1row[:])
          nc.vector.tensor_copy(s1bh[:], s1bf[:])

          # interleave: emit phase-1 segment s, then main pairs of segment s-1
          for s in range(1, SEG):
            phase1_segment(s)
            for g in range((s - 1) * NBS // 2, s * NBS // 2):
                main_pair(g)
          for g in range((SEG - 1) * NBS // 2, CH // 2):
            main_pair(g)

          if DEBUG_DUMPS:
            nc.sync.dma_start(dbg_s12, s12sb[:])
            nc.sync.dma_start(dbg_s1b, s1bh[:])
            nc.sync.dma_start(dbg_whe, whe[:, 0:2 * F])
            dent = opool.tile([128, 2 * IB], F32, tag="dent", name="dent")
            nc.vector.tensor_copy(dent[:], den[:])
            nc.sync.dma_start(dbg_den, dent[:])

          # normalize + store
          for ib in range(IB):
            rec = rcpool.tile([128, 1], F32, tag="rec", name="rec")
            if "no_den" in ABLATE:
                nc.vector.memset(rec[:], 1.0)
            else:
                nc.vector.reciprocal(rec[:], den[:, 2 * ib:2 * ib + 1])
            ot = opool.tile([128, F], F32, tag="ot", name="ot")
            acc_ap = (whe[:, ib * F:(ib + 1) * F] if "no_acc" in ABLATE else
                      accs[ib // 2][:, (ib % 2) * F:(ib % 2 + 1) * F])
            if NORM_ON_ACT:
                nc.scalar.activation(ot[:], acc_ap, AF.Copy, scale=rec[:])
            else:
                nc.vector.tensor_scalar_mul(ot[:], acc_ap, rec[:])
            nc.sync.dma_start(out[ib * 128:(ib + 1) * 128, :], ot[:])

    nc.compile()
    return nc


_CACHE = {}


def _get_nc(repeat=1):
    key = f"nc{repeat}"
    if key not in _CACHE:
        _CACHE[key] = _build(repeat)
    return _CACHE[key]


def kernel(adj, x, W, a):
    adj = np.asarray(adj, dtype=np.float32)
    x = np.asarray(x, dtype=np.float32)
    W = np.asarray(W, dtype=np.float32)
    a = np.asarray(a, dtype=np.float32)

    Wh = np.ascontiguousarray(W.astype(np.float16))
    WTc = np.ascontiguousarray(W.T.astype(np.float16))
    a12 = np.ascontiguousarray(
        np.stack([a[:F, 0], a[F:, 0]], axis=1).astype(np.float16))  # [F, 2]
    idx = np.arange(R)

    in_maps = []
    for c in range(NCORES):
        shift = c * R
        xT = np.ascontiguousarray(np.roll(x, -shift, axis=0).T.astype(np.float16))
        rows = np.roll(adj[shift:shift + R, :], -shift, axis=1)  # [R, N]
        rows[idx, idx] = 1.0                                     # self loops
        madjT = np.ascontiguousarray(
            np.where(rows > 0, np.float16(0.0), np.float16(-BIG)).T)
        in_maps.append({"xT": xT, "W": Wh, "WT": WTc, "a12": a12,
                        "madjT": madjT})

    res = run_bass_kernel_spmd(_get_nc(), in_maps, list(range(NCORES)))
    return np.concatenate([r["out"] for r in res.results], axis=0)


if __name__ == "__main__":
    rng = np.random.default_rng(0)
    adj = (rng.integers(0, 2, (N, N))).astype(np.float32)
    x = rng.normal(size=(N, F)).astype(np.float32)
    W = rng.normal(size=(F, F)).astype(np.float32) * 0.1
    a = rng.normal(size=(2 * F, 1)).astype(np.float32) * 0.1
    out = kernel(adj, x, W, a)
    print(out.shape, out.dtype)
